# revision 1
# baseline (speedup 1.0000x reference)
"""Trainium2 Bass kernel for the 1D differentiable Euler solver (Roe flux,
Harten entropy fix, CFL-adaptive dt, 32 first-order steps).

Strategy (8 NeuronCores, SPMD):
  - Shard the 1,048,576-cell grid spatially: 131,072 cells/core laid out as
    [128 partitions x 1024 cells], plus G=32 ghost cells per partition side
    (host gathers overlapping, edge-clamped windows). With G >= n_steps each
    partition advances the full time loop with no per-step neighbor
    exchange; per-step work is pure elementwise DVE/ACT ops on [128, 1088]
    tiles held entirely in SBUF.
  - The only global coupling is the CFL dt = CFL*DX / max(|u|+c): a [128,1]
    per-partition max goes through a tiny AllReduce(max) across the 8 cores
    each step, overlapped with the interface-flux computation; a GPSIMD
    partition_all_reduce then folds+broadcasts it to every partition.
  - Stale ghost columns are re-filled each step from the nearest valid
    column ("sanitize"), and the two global-edge rows re-clamp their ghosts
    via masked predicated copies, so every lane always holds physical data
    and the local max never sees garbage.

kernel(**inputs) takes the FULL unsharded inputs and returns full
(rho, u, p) float32 arrays, matching reference.reference().
"""

import numpy as np

import concourse.bass as bass
import concourse.bacc as bacc
import concourse.tile as tile
import concourse.mybir as mybir
from concourse import bass_isa
from concourse.bass_utils import run_bass_kernel_spmd

F32 = mybir.dt.float32
U8 = mybir.dt.uint8
ALU = mybir.AluOpType
ACTF = mybir.ActivationFunctionType
AX = mybir.AxisListType

GAMMA = 1.4
CFL = 0.5
DX = 1e-3

NX = 1048576
NC = 8
P = 128
FPC = NX // NC // P          # 1024 cells per partition
G = 32                       # ghost width per side (>= n_steps)
W = FPC + 2 * G              # 1088 columns per partition
V = W - 1                    # interfaces per partition row

_CACHE = {}
_last_results = None


def _build(n_steps: int):
    """Build + compile the SPMD program for a given unrolled step count."""
    nc = bacc.Bacc("TRN2", target_bir_lowering=False, debug=False,
                   enable_asserts=False, num_devices=NC)

    rho_in = nc.dram_tensor("rho_in", [P, W], F32, kind="ExternalInput")
    mu_in = nc.dram_tensor("mu_in", [P, W], F32, kind="ExternalInput")
    E_in = nc.dram_tensor("E_in", [P, W], F32, kind="ExternalInput")
    tf_in = nc.dram_tensor("tf_in", [1, 1], F32, kind="ExternalInput")
    mskL_in = nc.dram_tensor("mskL_in", [P, G], U8, kind="ExternalInput")
    mskR_in = nc.dram_tensor("mskR_in", [P, G], U8, kind="ExternalInput")
    rho_out = nc.dram_tensor("rho_out", [P, FPC], F32, kind="ExternalOutput")
    u_out = nc.dram_tensor("u_out", [P, FPC], F32, kind="ExternalOutput")
    p_out = nc.dram_tensor("p_out", [P, FPC], F32, kind="ExternalOutput")

    with tile.TileContext(nc) as tc:
        with (
            tc.tile_pool(name="sb", bufs=1) as sb,
            tc.tile_pool(name="dram", bufs=1, space="DRAM") as dram,
        ):
            # persistent state
            rho = sb.tile([P, W], F32, tag="rho", name="rho")
            mu = sb.tile([P, W], F32, tag="mu", name="mu")
            En = sb.tile([P, W], F32, tag="En", name="En")

            # work buffers, managed by a tiny liveness allocator
            NWORK = 30
            wk = [sb.tile([P, W], F32, tag=f"wk{i}", name=f"wk{i}") for i in range(NWORK)]
            free = list(wk)
            live = {}

            def get(name):
                t = free.pop()
                live[name] = t
                return t

            def rel(*names):
                for n in names:
                    free.append(live.pop(n))

            # small tiles
            mskL = sb.tile([P, G], U8, tag="mskL", name="mskL")
            mskR = sb.tile([P, G], U8, tag="mskR", name="mskR")
            wmax = sb.tile([P, 1], F32, tag="wmax", name="wmax")
            gpp = sb.tile([P, 1], F32, tag="gpp", name="gpp")
            gball = sb.tile([P, 1], F32, tag="gball", name="gball")
            rgi = sb.tile([P, 1], F32, tag="rgi", name="rgi")
            rgs = sb.tile([P, 1], F32, tag="rgs", name="rgs")
            dt0 = sb.tile([P, 1], F32, tag="dt0", name="dt0")
            rem = sb.tile([P, 1], F32, tag="rem", name="rem")
            dtt = sb.tile([P, 1], F32, tag="dtt", name="dtt")
            tcur = sb.tile([P, 1], F32, tag="tcur", name="tcur")
            hdtn = sb.tile([P, 1], F32, tag="hdtn", name="hdtn")
            tf1 = sb.tile([1, 1], F32, tag="tf1", name="tf1")
            tfb = sb.tile([P, 1], F32, tag="tfb", name="tfb")

            cc_in = dram.tile([P, 1], F32, tag="cc_in", name="cc_in")
            cc_out = dram.tile([P, 1], F32, tag="cc_out", name="cc_out")

            vec = nc.vector
            act = nc.scalar
            gps = nc.gpsimd

            # ---- prologue ----
            nc.sync.dma_start(out=rho[:], in_=rho_in.ap())
            nc.sync.dma_start(out=mu[:], in_=mu_in.ap())
            nc.sync.dma_start(out=En[:], in_=E_in.ap())
            nc.sync.dma_start(out=mskL[:], in_=mskL_in.ap())
            nc.sync.dma_start(out=mskR[:], in_=mskR_in.ap())
            nc.sync.dma_start(out=tf1[:], in_=tf_in.ap())
            gps.partition_broadcast(tfb[:], tf1[:])
            vec.memset(tcur[:], 0.0)

            for s in range(n_steps):
                state3 = ((rho, "r"), (mu, "m"), (En, "e"))
                if s > 0:
                    # sanitize stale columns from nearest valid column
                    for st, _nm in state3:
                        act.copy(st[:, 0:s], st[:, s:s + 1].broadcast_to((P, s)))
                        act.copy(st[:, W - s:W],
                                 st[:, W - s - 1:W - s].broadcast_to((P, s)))
                    # re-clamp global-edge ghosts (masked; mask is per-core data)
                    for st, _nm in state3:
                        vec.copy_predicated(st[:, 0:G], mskL[:],
                                            st[:, G:G + 1].broadcast_to((P, G)))
                        vec.copy_predicated(st[:, W - G:W], mskR[:],
                                            st[:, W - G - 1:W - G].broadcast_to((P, G)))

                # ---- stage A: cell-centered quantities (full width W) ----
                sc0 = get("sc0")
                rinv = get("rinv")
                vec.reciprocal_approx_accurate(rinv[:], rho[:], sc0[:])
                rel("sc0")
                u = get("u")
                vec.tensor_tensor(u[:], mu[:], rinv[:], ALU.mult)
                q = get("q")
                vec.tensor_tensor(q[:], mu[:], u[:], ALU.mult)
                E4 = get("E4")
                vec.tensor_scalar_mul(E4[:], En[:], 0.4)
                p = get("p")
                vec.scalar_tensor_tensor(p[:], q[:], -0.2, E4[:], ALU.mult, ALU.add)
                Fm = get("Fm")
                vec.scalar_tensor_tensor(Fm[:], q[:], 0.8, E4[:], ALU.mult, ALU.add)
                rel("q", "E4")
                Ep = get("Ep")
                vec.tensor_tensor(Ep[:], En[:], p[:], ALU.add)
                pr = get("pr")
                vec.tensor_tensor(pr[:], p[:], rinv[:], ALU.mult)
                cc = get("cc")
                act.activation(cc[:], pr[:], ACTF.Sqrt, scale=float(GAMMA))
                rel("pr")
                sq = get("sq")
                act.activation(sq[:], rho[:], ACTF.Sqrt)
                irs = get("irs")
                vec.tensor_tensor(irs[:], rinv[:], sq[:], ALU.mult)
                rel("rinv")
                sH = get("sH")
                vec.tensor_tensor(sH[:], Ep[:], irs[:], ALU.mult)
                rel("irs")
                su = get("su")
                vec.tensor_tensor(su[:], sq[:], u[:], ALU.mult)
                Fe = get("Fe")
                vec.tensor_tensor(Fe[:], u[:], Ep[:], ALU.mult)
                rel("Ep")
                au = get("au")
                act.activation(au[:], u[:], ACTF.Abs)
                wsc = get("wsc")
                vec.tensor_tensor(wsc[:], au[:], cc[:], ALU.add)
                vec.tensor_reduce(wmax[:], wsc[:], axis=AX.X, op=ALU.max)
                rel("au", "wsc")

                # ---- dt: tiny AllReduce(max) overlapped with stage B ----
                nc.sync.dma_start(out=cc_in[:], in_=wmax[:])
                gps.collective_compute(
                    "AllReduce", ALU.max,
                    replica_groups=[list(range(NC))],
                    ins=[cc_in[:]], outs=[cc_out[:]])
                nc.sync.dma_start(out=gpp[:], in_=cc_out[:])
                gps.partition_all_reduce(gball[:], gpp[:], channels=P,
                                         reduce_op=bass_isa.ReduceOp.max)
                vec.reciprocal_approx_accurate(rgi[:], gball[:], rgs[:])
                vec.tensor_scalar_mul(dt0[:], rgi[:], float(CFL * DX))
                vec.scalar_tensor_tensor(rem[:], tcur[:], -1.0, tfb[:],
                                         ALU.mult, ALU.add)
                vec.tensor_scalar_max(rem[:], rem[:], 0.0)
                vec.tensor_tensor(dtt[:], dt0[:], rem[:], ALU.min)
                vec.tensor_tensor(tcur[:], tcur[:], dtt[:], ALU.add)
                vec.tensor_scalar_mul(hdtn[:], dtt[:], float(-0.5 / DX))

                # ---- stage B: interface quantities (width V = W-1) ----
                def Ls(t):
                    return t[:, 0:V]

                def Rs(t):
                    return t[:, 1:W]

                den = get("den")
                vec.tensor_tensor(den[:, 0:V], Ls(sq), Rs(sq), ALU.add)
                sc1 = get("sc1")
                dinv = get("dinv")
                vec.reciprocal_approx_accurate(dinv[:, 0:V], den[:, 0:V],
                                               sc1[:, 0:V])
                rel("sc1", "den")
                ur = get("ur")
                vec.tensor_tensor(ur[:, 0:V], Ls(su), Rs(su), ALU.add)
                vec.tensor_tensor(ur[:, 0:V], ur[:, 0:V], dinv[:, 0:V], ALU.mult)
                rel("su")
                Hr = get("Hr")
                vec.tensor_tensor(Hr[:, 0:V], Ls(sH), Rs(sH), ALU.add)
                vec.tensor_tensor(Hr[:, 0:V], Hr[:, 0:V], dinv[:, 0:V], ALU.mult)
                rel("sH", "dinv")
                ur2 = get("ur2")
                act.activation(ur2[:, 0:V], ur[:, 0:V], ACTF.Square)
                d = get("d")
                vec.scalar_tensor_tensor(d[:, 0:V], ur2[:, 0:V], -0.5, Hr[:, 0:V],
                                         ALU.mult, ALU.add)
                cr = get("cr")
                act.activation(cr[:, 0:V], d[:, 0:V], ACTF.Sqrt,
                               scale=float(GAMMA - 1.0))
                e2 = get("e2")
                vec.tensor_scalar_mul(e2[:, 0:V], d[:, 0:V],
                                      float(0.01 * (GAMMA - 1.0)))
                tc2 = get("tc2")
                vec.tensor_scalar_mul(tc2[:, 0:V], d[:, 0:V],
                                      float(2.0 * (GAMMA - 1.0)))
                sc2 = get("sc2")
                ic2h = get("ic2h")
                vec.reciprocal_approx_accurate(ic2h[:, 0:V], tc2[:, 0:V],
                                               sc2[:, 0:V])
                rel("sc2", "tc2")
                l1 = get("l1")
                vec.tensor_tensor(l1[:, 0:V], ur[:, 0:V], cr[:, 0:V], ALU.subtract)
                l3 = get("l3")
                vec.tensor_tensor(l3[:, 0:V], ur[:, 0:V], cr[:, 0:V], ALU.add)
                q1 = get("q1")
                act.activation(q1[:, 0:V], l1[:, 0:V], ACTF.Square)
                rel("l1")
                q3 = get("q3")
                act.activation(q3[:, 0:V], l3[:, 0:V], ACTF.Square)
                rel("l3")
                vec.tensor_tensor(q1[:, 0:V], q1[:, 0:V], e2[:, 0:V], ALU.add)
                vec.tensor_tensor(q3[:, 0:V], q3[:, 0:V], e2[:, 0:V], ALU.add)
                a2t = get("a2t")
                vec.tensor_tensor(a2t[:, 0:V], ur2[:, 0:V], e2[:, 0:V], ALU.add)
                rel("ur2", "e2")
                a1 = get("a1")
                act.activation(a1[:, 0:V], q1[:, 0:V], ACTF.Sqrt)
                rel("q1")
                a2 = get("a2")
                act.activation(a2[:, 0:V], a2t[:, 0:V], ACTF.Sqrt)
                rel("a2t")
                a3 = get("a3")
                act.activation(a3[:, 0:V], q3[:, 0:V], ACTF.Sqrt)
                rel("q3")
                drho = get("drho")
                vec.tensor_tensor(drho[:, 0:V], Rs(rho), Ls(rho), ALU.subtract)
                dp = get("dp")
                vec.tensor_tensor(dp[:, 0:V], Rs(p), Ls(p), ALU.subtract)
                rel("p")
                du = get("du")
                vec.tensor_tensor(du[:, 0:V], Rs(u), Ls(u), ALU.subtract)
                rel("u")
                crdu = get("crdu")
                vec.tensor_tensor(crdu[:, 0:V], Rs(rho), du[:, 0:V], ALU.mult)
                rel("du")
                vec.tensor_tensor(crdu[:, 0:V], cr[:, 0:V], crdu[:, 0:V], ALU.mult)
                x1 = get("x1")
                vec.tensor_tensor(x1[:, 0:V], dp[:, 0:V], crdu[:, 0:V],
                                  ALU.subtract)
                x3 = get("x3")
                vec.tensor_tensor(x3[:, 0:V], dp[:, 0:V], crdu[:, 0:V], ALU.add)
                rel("crdu")
                vec.tensor_tensor(x1[:, 0:V], a1[:, 0:V], x1[:, 0:V], ALU.mult)
                vec.tensor_tensor(x3[:, 0:V], a3[:, 0:V], x3[:, 0:V], ALU.mult)
                rel("a1", "a3")
                bp = get("bp")
                vec.tensor_tensor(bp[:, 0:V], x1[:, 0:V], x3[:, 0:V], ALU.add)
                bm = get("bm")
                vec.tensor_tensor(bm[:, 0:V], x3[:, 0:V], x1[:, 0:V], ALU.subtract)
                rel("x1", "x3")
                m2 = get("m2")
                vec.scalar_tensor_tensor(m2[:, 0:V], dp[:, 0:V], 2.0,
                                         ic2h[:, 0:V], ALU.mult, ALU.mult)
                rel("dp")
                vec.tensor_tensor(m2[:, 0:V], drho[:, 0:V], m2[:, 0:V],
                                  ALU.subtract)
                rel("drho")
                G2 = get("G2")
                vec.tensor_tensor(G2[:, 0:V], a2[:, 0:V], m2[:, 0:V], ALU.mult)
                rel("a2", "m2")
                Sp = get("Sp")
                vec.tensor_tensor(Sp[:, 0:V], bp[:, 0:V], ic2h[:, 0:V], ALU.mult)
                rel("bp")
                Sm = get("Sm")
                vec.tensor_tensor(Sm[:, 0:V], bm[:, 0:V], ic2h[:, 0:V], ALU.mult)
                rel("bm", "ic2h")
                dr = get("dr")
                vec.tensor_tensor(dr[:, 0:V], Sp[:, 0:V], G2[:, 0:V], ALU.add)
                rel("Sp")
                csm = get("csm")
                vec.tensor_tensor(csm[:, 0:V], cr[:, 0:V], Sm[:, 0:V], ALU.mult)
                rel("cr", "Sm")
                dm = get("dm")
                vec.tensor_tensor(dm[:, 0:V], ur[:, 0:V], dr[:, 0:V], ALU.mult)
                vec.tensor_tensor(dm[:, 0:V], dm[:, 0:V], csm[:, 0:V], ALU.add)
                w1 = get("w1")
                vec.tensor_tensor(w1[:, 0:V], Hr[:, 0:V], dr[:, 0:V], ALU.mult)
                rel("Hr")
                w2 = get("w2")
                vec.tensor_tensor(w2[:, 0:V], d[:, 0:V], G2[:, 0:V], ALU.mult)
                rel("d", "G2")
                w3 = get("w3")
                vec.tensor_tensor(w3[:, 0:V], ur[:, 0:V], csm[:, 0:V], ALU.mult)
                rel("ur", "csm")
                vec.tensor_tensor(w1[:, 0:V], w1[:, 0:V], w2[:, 0:V], ALU.subtract)
                rel("w2")
                de = get("de")
                vec.tensor_tensor(de[:, 0:V], w1[:, 0:V], w3[:, 0:V], ALU.add)
                rel("w1", "w3")

                # ---- fluxes + update ----
                Pr = get("Pr")
                vec.tensor_tensor(Pr[:, 0:V], Ls(mu), Rs(mu), ALU.add)
                vec.tensor_tensor(Pr[:, 0:V], Pr[:, 0:V], dr[:, 0:V],
                                  ALU.subtract)
                rel("dr")
                Pm = get("Pm")
                vec.tensor_tensor(Pm[:, 0:V], Ls(Fm), Rs(Fm), ALU.add)
                vec.tensor_tensor(Pm[:, 0:V], Pm[:, 0:V], dm[:, 0:V],
                                  ALU.subtract)
                rel("Fm", "dm")
                Pe = get("Pe")
                vec.tensor_tensor(Pe[:, 0:V], Ls(Fe), Rs(Fe), ALU.add)
                vec.tensor_tensor(Pe[:, 0:V], Pe[:, 0:V], de[:, 0:V],
                                  ALU.subtract)
                rel("Fe", "de")

                UPD = W - 2  # cells 1..W-2 get updated
                for Phi_name, st in (("Pr", rho), ("Pm", mu), ("Pe", En)):
                    Phi = live[Phi_name]
                    dPhi = get("dPhi")
                    vec.tensor_tensor(dPhi[:, 0:UPD], Phi[:, 1:V],
                                      Phi[:, 0:V - 1], ALU.subtract)
                    vec.scalar_tensor_tensor(st[:, 1:W - 1], dPhi[:, 0:UPD],
                                             hdtn[:], st[:, 1:W - 1],
                                             ALU.mult, ALU.add)
                    rel("dPhi", Phi_name)

                rel("sq")
                rel("cc")
                assert len(free) == NWORK, (s, len(free), list(live))

            # ---- epilogue: final u, p on own cells; store ----
            sc0 = get("sc0")
            rinv = get("rinv")
            vec.reciprocal_approx_accurate(rinv[:], rho[:], sc0[:])
            u = get("u")
            vec.tensor_tensor(u[:], mu[:], rinv[:], ALU.mult)
            q = get("q")
            vec.tensor_tensor(q[:], mu[:], u[:], ALU.mult)
            E4 = get("E4")
            vec.tensor_scalar_mul(E4[:], En[:], 0.4)
            p = get("p")
            vec.scalar_tensor_tensor(p[:], q[:], -0.2, E4[:], ALU.mult, ALU.add)
            own = slice(G, G + FPC)
            nc.sync.dma_start(out=rho_out.ap(), in_=rho[:, own])
            nc.sync.dma_start(out=u_out.ap(), in_=u[:, own])
            nc.sync.dma_start(out=p_out.ap(), in_=p[:, own])

    nc.compile()
    return nc


def _get_program(n_steps: int):
    if n_steps not in _CACHE:
        _CACHE[n_steps] = _build(n_steps)
    return _CACHE[n_steps]


def kernel(rho_init, u_init, p_init, t_final, n_steps):
    rho_init = np.ascontiguousarray(np.asarray(rho_init, np.float32))
    u_init = np.ascontiguousarray(np.asarray(u_init, np.float32))
    p_init = np.ascontiguousarray(np.asarray(p_init, np.float32))
    tf = np.float32(np.asarray(t_final).reshape(()))
    ns = int(np.asarray(n_steps).reshape(()))
    assert rho_init.shape == (NX,)

    gm1 = np.float32(GAMMA - 1.0)
    cells = NX // NC
    idx = (np.arange(P)[:, None] * FPC) + (np.arange(W)[None, :] - G)

    in_maps = []
    for k in range(NC):
        gi = np.clip(k * cells + idx, 0, NX - 1)
        r = rho_init[gi]
        u = u_init[gi]
        p = p_init[gi]
        mu = r * u
        E = p / gm1 + np.float32(0.5) * r * u * u
        mskL = np.zeros((P, G), np.uint8)
        mskR = np.zeros((P, G), np.uint8)
        if k == 0:
            mskL[0, :] = 1
        if k == NC - 1:
            mskR[P - 1, :] = 1
        in_maps.append({
            "rho_in": np.ascontiguousarray(r),
            "mu_in": np.ascontiguousarray(mu),
            "E_in": np.ascontiguousarray(E),
            "tf_in": np.full((1, 1), tf, np.float32),
            "mskL_in": mskL,
            "mskR_in": mskR,
        })

    nc = _get_program(ns)
    res = run_bass_kernel_spmd(nc, in_maps, core_ids=list(range(NC)))
    global _last_results
    _last_results = res

    rho_o = np.empty(NX, np.float32)
    u_o = np.empty(NX, np.float32)
    p_o = np.empty(NX, np.float32)
    for k in range(NC):
        sl = slice(k * cells, (k + 1) * cells)
        rho_o[sl] = res.results[k]["rho_out"].reshape(-1)
        u_o[sl] = res.results[k]["u_out"].reshape(-1)
        p_o[sl] = res.results[k]["p_out"].reshape(-1)
    return rho_o, u_o, p_o



# revision 3
# speedup vs baseline: 1.4999x; 1.4999x over previous
"""Trainium2 Bass kernel for the 1D differentiable Euler solver (Roe flux,
Harten entropy fix, CFL-adaptive dt, 32 first-order steps).

Strategy (8 NeuronCores, SPMD):
  - Spatial shard: 131,072 cells/core as [128 partitions x 1024 cells] plus
    G=32 edge-clamped ghost cells per partition side (G >= n_steps), so the
    whole time loop runs from SBUF with no per-step halo DMA.
  - Mixed precision: state + central fluxes in fp32; the Roe dissipation
    chain in fp16 (DVE 2x mode), with scale factors folded into ACT
    activation scales and tensor_scalar immediates.
  - Engine split: DVE does the tensor-tensor work (fp16 2x), ACT does all
    casts/sqrt/square/abs, GPSIMD does the fp32 central-flux ops and the
    cell-centered flux differences.
  - dt = CFL*DX/max(|u|+c): per-core [P,1] max -> AllReduce(max) kicked off
    mid-step and consumed only after the whole dissipation chain is queued,
    so the collective latency hides behind the stage-B DVE work.

kernel(**inputs) takes FULL unsharded inputs, returns full (rho, u, p).
"""

import numpy as np

import concourse.bass as bass
import concourse.bacc as bacc
import concourse.tile as tile
import concourse.mybir as mybir
from concourse import bass_isa
from concourse.bass_utils import run_bass_kernel_spmd

F32 = mybir.dt.float32
F16 = mybir.dt.float16
U8 = mybir.dt.uint8
ALU = mybir.AluOpType
ACTF = mybir.ActivationFunctionType
AX = mybir.AxisListType

GAMMA = 1.4
CFL = 0.5
DX = 1e-3

NX = 1048576
NC = 8
P = 128
FPC = NX // NC // P          # 1024 cells per partition
G = 32                       # ghost width per side (>= n_steps)
W = FPC + 2 * G              # 1088 columns per partition
V = W - 1                    # interfaces per partition row
UPD = W - 2                  # updated cells per partition row

_CACHE = {}
_last_results = None


def _build(n_steps: int):
    nc = bacc.Bacc("TRN2", target_bir_lowering=False, debug=False,
                   enable_asserts=False, num_devices=NC)

    rho_in = nc.dram_tensor("rho_in", [P, W], F32, kind="ExternalInput")
    mu_in = nc.dram_tensor("mu_in", [P, W], F32, kind="ExternalInput")
    E_in = nc.dram_tensor("E_in", [P, W], F32, kind="ExternalInput")
    tf_in = nc.dram_tensor("tf_in", [1, 1], F32, kind="ExternalInput")
    mskL_in = nc.dram_tensor("mskL_in", [P, G], U8, kind="ExternalInput")
    mskR_in = nc.dram_tensor("mskR_in", [P, G], U8, kind="ExternalInput")
    rho_out = nc.dram_tensor("rho_out", [P, FPC], F32, kind="ExternalOutput")
    u_out = nc.dram_tensor("u_out", [P, FPC], F32, kind="ExternalOutput")
    p_out = nc.dram_tensor("p_out", [P, FPC], F32, kind="ExternalOutput")

    with tile.TileContext(nc) as tc:
        with (
            tc.tile_pool(name="sb", bufs=1) as sb,
            tc.tile_pool(name="dram", bufs=1, space="DRAM") as dram,
        ):
            def t32(name):
                return sb.tile([P, W], F32, tag=name, name=name)

            def t16(name):
                return sb.tile([P, W], F16, tag=name, name=name)

            # persistent fp32 state
            rho, mu, En = t32("rho"), t32("mu"), t32("En")
            # fp32 work
            rinv, uu, q, p04, pp = (t32(n) for n in
                                    ("rinv", "uu", "q", "p04", "pp"))
            Ep, Fm, Fe = t32("Ep"), t32("Fm"), t32("Fe")
            c32a, c32b = t32("c32a"), t32("c32b")
            # fp16 per-cell feeders
            u16, rinv16, Ep16, rh25, p25 = (t16(n) for n in
                                            ("u16", "rinv16", "Ep16",
                                             "rh25", "p25"))
            sq16, irs2, au16, cc16 = (t16(n) for n in
                                      ("sq16", "irs2", "au16", "cc16"))
            pr25, su16, sH2 = t16("pr25"), t16("su16"), t16("sH2")
            # fp16 interface chain
            den16, dinv16, urr, Hr2, D2 = (t16(n) for n in
                                           ("den16", "dinv16", "urr",
                                            "Hr2", "D2"))
            ur2f, e2f, rd16, rdX, cr = (t16(n) for n in
                                        ("ur2f", "e2f", "rd16", "rdX", "cr"))
            l1, l3, s1, s3, a2t = (t16(n) for n in
                                   ("l1", "l3", "s1", "s3", "a2t"))
            a1s, a2s, a3s = t16("a1s"), t16("a2s"), t16("a3s")
            du, dpd, drd = t16("du"), t16("dpd"), t16("drd")
            crdu, X1, X3 = t16("crdu"), t16("X1"), t16("X3")
            bp, bm, mtt, M2 = t16("bp"), t16("bm"), t16("mtt"), t16("M2")
            G2, Sp, Sm, dr = t16("G2"), t16("Sp"), t16("Sm"), t16("dr")
            csm, dm, dSp, DE2 = t16("csm"), t16("dm"), t16("dSp"), t16("DE2")
            gg = [t16(f"gg{i}") for i in range(3)]
            ddt = t16("ddt")
            wsc = t16("wsc")

            # small tiles
            mskL = sb.tile([P, G], U8, tag="mskL", name="mskL")
            mskR = sb.tile([P, G], U8, tag="mskR", name="mskR")
            small = {}
            for n in ("wmax", "gpp", "gball", "rgi", "rgs", "dt0", "rem",
                      "dtt", "tcur", "hdtn", "tfb"):
                small[n] = sb.tile([P, 1], F32, tag=n, name=n)
            tf1 = sb.tile([1, 1], F32, tag="tf1", name="tf1")

            cc_in = dram.tile([P, 1], F32, tag="cc_in", name="cc_in")
            cc_out = dram.tile([P, 1], F32, tag="cc_out", name="cc_out")

            vec = nc.vector
            act = nc.scalar
            gps = nc.gpsimd

            # ---- prologue ----
            nc.sync.dma_start(out=rho[:], in_=rho_in.ap())
            nc.sync.dma_start(out=mu[:], in_=mu_in.ap())
            nc.sync.dma_start(out=En[:], in_=E_in.ap())
            nc.sync.dma_start(out=mskL[:], in_=mskL_in.ap())
            nc.sync.dma_start(out=mskR[:], in_=mskR_in.ap())
            nc.sync.dma_start(out=tf1[:], in_=tf_in.ap())
            gps.partition_broadcast(small["tfb"][:], tf1[:])
            vec.memset(small["tcur"][:], 0.0)

            def Ls(t):
                return t[:, 0:V]

            def Rs(t):
                return t[:, 1:W]

            for s in range(n_steps):
                state3 = (rho, mu, En)
                if s > 0:
                    for st in state3:
                        act.copy(st[:, 0:s], st[:, s:s + 1].broadcast_to((P, s)))
                        act.copy(st[:, W - s:W],
                                 st[:, W - s - 1:W - s].broadcast_to((P, s)))
                    for st in state3:
                        vec.copy_predicated(st[:, 0:G], mskL[:],
                                            st[:, G:G + 1].broadcast_to((P, G)))
                        vec.copy_predicated(st[:, W - G:W], mskR[:],
                                            st[:, W - G - 1:W - G].broadcast_to((P, G)))

                # ---- stage A: cell-centered (width W) ----
                vec.reciprocal_approx_fast(rinv[:], rho[:])
                vec.tensor_tensor(uu[:], mu[:], rinv[:], ALU.mult)
                vec.tensor_tensor(q[:], mu[:], uu[:], ALU.mult)
                # p04 = E - q/2 = p/(g-1) = 2.5*p ; p = 0.4*p04
                vec.scalar_tensor_tensor(p04[:], q[:], -0.5, En[:],
                                         ALU.mult, ALU.add)
                vec.tensor_scalar_mul(pp[:], p04[:], 0.4)

                # gps: central-flux building blocks (fp32)
                gps.tensor_tensor(Ep[:], En[:], pp[:], ALU.add)
                gps.tensor_tensor(Fm[:], q[:], pp[:], ALU.add)
                gps.tensor_tensor(Fe[:], uu[:], Ep[:], ALU.mult)

                # act: fp16 feeders (scale folds free)
                act.copy(u16[:], uu[:])
                act.copy(rinv16[:], rinv[:])
                act.copy(p25[:], p04[:])
                act.mul(rh25[:], rho[:], 2.5)
                act.activation(sq16[:], rho[:], ACTF.Sqrt)
                act.activation(irs2[:], rinv[:], ACTF.Sqrt, scale=4.0)
                act.copy(Ep16[:], Ep[:])
                act.activation(au16[:], uu[:], ACTF.Abs)

                # vec: fp16 feeders
                vec.tensor_tensor(pr25[:], p25[:], rinv16[:], ALU.mult)
                act.activation(cc16[:], pr25[:], ACTF.Sqrt, scale=0.56)
                vec.tensor_tensor(su16[:], sq16[:], u16[:], ALU.mult)
                vec.tensor_tensor(sH2[:], Ep16[:], irs2[:], ALU.mult)

                # wave-speed max over own cells
                own = slice(G, G + FPC)
                vec.tensor_tensor(wsc[:, 0:FPC], au16[:, own], cc16[:, own],
                                  ALU.add)
                vec.tensor_reduce(small["wmax"][:], wsc[:, 0:FPC],
                                  axis=AX.X, op=ALU.max)

                # kick off the dt AllReduce now; consume after stage B
                nc.sync.dma_start(out=cc_in[:], in_=small["wmax"][:])
                gps.collective_compute(
                    "AllReduce", ALU.max,
                    replica_groups=[list(range(NC))],
                    ins=[cc_in[:]], outs=[cc_out[:]])
                nc.sync.dma_start(out=small["gpp"][:], in_=cc_out[:])

                # ---- stage B: interface chain (fp16, width V) ----
                vec.tensor_tensor(den16[:, 0:V], Ls(sq16), Rs(sq16), ALU.add)
                act.copy(c32a[:, 0:V], den16[:, 0:V])
                vec.reciprocal_approx_fast(c32b[:, 0:V], c32a[:, 0:V])
                act.copy(dinv16[:, 0:V], c32b[:, 0:V])
                vec.tensor_tensor(urr[:, 0:V], Ls(su16), Rs(su16), ALU.add)
                vec.tensor_tensor(urr[:, 0:V], urr[:, 0:V], dinv16[:, 0:V],
                                  ALU.mult)
                vec.tensor_tensor(Hr2[:, 0:V], Ls(sH2), Rs(sH2), ALU.add)
                vec.tensor_tensor(Hr2[:, 0:V], Hr2[:, 0:V], dinv16[:, 0:V],
                                  ALU.mult)
                act.activation(ur2f[:, 0:V], urr[:, 0:V], ACTF.Square)
                # D2 = 2d = 2H - u^2 ; c^2 = 0.2*D2
                vec.tensor_tensor(D2[:, 0:V], Hr2[:, 0:V], ur2f[:, 0:V],
                                  ALU.subtract)
                act.activation(cr[:, 0:V], D2[:, 0:V], ACTF.Sqrt, scale=0.2)
                # rd16 = 1/(2c^2) = 2.5/D2
                act.copy(c32a[:, 0:V], D2[:, 0:V])
                vec.reciprocal_approx_fast(c32b[:, 0:V], c32a[:, 0:V])
                act.mul(rd16[:, 0:V], c32b[:, 0:V], 2.5)
                # eps^2 = 0.01 c^2 = 0.002*D2
                vec.tensor_scalar_mul(e2f[:, 0:V], D2[:, 0:V], 0.002)
                vec.tensor_tensor(l1[:, 0:V], urr[:, 0:V], cr[:, 0:V],
                                  ALU.subtract)
                vec.tensor_tensor(l3[:, 0:V], urr[:, 0:V], cr[:, 0:V],
                                  ALU.add)
                act.activation(s1[:, 0:V], l1[:, 0:V], ACTF.Square)
                act.activation(s3[:, 0:V], l3[:, 0:V], ACTF.Square)
                vec.tensor_tensor(s1[:, 0:V], s1[:, 0:V], e2f[:, 0:V], ALU.add)
                vec.tensor_tensor(s3[:, 0:V], s3[:, 0:V], e2f[:, 0:V], ALU.add)
                vec.tensor_tensor(a2t[:, 0:V], ur2f[:, 0:V], e2f[:, 0:V],
                                  ALU.add)
                # a1s = 0.4*a1, a3s = 0.4*a3 (folds the 1/2.5 wave scaling)
                act.activation(a1s[:, 0:V], s1[:, 0:V], ACTF.Sqrt, scale=0.16)
                act.activation(a3s[:, 0:V], s3[:, 0:V], ACTF.Sqrt, scale=0.16)
                act.activation(a2s[:, 0:V], a2t[:, 0:V], ACTF.Sqrt)
                # diffs (fp32 in -> fp16 out for accuracy)
                vec.tensor_tensor(du[:, 0:V], Rs(uu), Ls(uu), ALU.subtract)
                vec.tensor_tensor(dpd[:, 0:V], Rs(p04), Ls(p04), ALU.subtract)
                vec.tensor_tensor(drd[:, 0:V], Rs(rho), Ls(rho), ALU.subtract)
                # crdu = 2.5*c*rhoR*du
                vec.tensor_tensor(crdu[:, 0:V], Rs(rh25), du[:, 0:V], ALU.mult)
                vec.tensor_tensor(crdu[:, 0:V], crdu[:, 0:V], cr[:, 0:V],
                                  ALU.mult)
                vec.tensor_tensor(X1[:, 0:V], dpd[:, 0:V], crdu[:, 0:V],
                                  ALU.subtract)
                vec.tensor_tensor(X3[:, 0:V], dpd[:, 0:V], crdu[:, 0:V],
                                  ALU.add)
                vec.tensor_tensor(X1[:, 0:V], a1s[:, 0:V], X1[:, 0:V],
                                  ALU.mult)
                vec.tensor_tensor(X3[:, 0:V], a3s[:, 0:V], X3[:, 0:V],
                                  ALU.mult)
                vec.tensor_tensor(bp[:, 0:V], X1[:, 0:V], X3[:, 0:V], ALU.add)
                vec.tensor_tensor(bm[:, 0:V], X3[:, 0:V], X1[:, 0:V],
                                  ALU.subtract)
                # al2 = drho - 0.8*dp25*rd
                vec.tensor_scalar_mul(rdX[:, 0:V], rd16[:, 0:V], 0.8)
                vec.tensor_tensor(mtt[:, 0:V], dpd[:, 0:V], rdX[:, 0:V],
                                  ALU.mult)
                vec.tensor_tensor(M2[:, 0:V], drd[:, 0:V], mtt[:, 0:V],
                                  ALU.subtract)
                vec.tensor_tensor(G2[:, 0:V], a2s[:, 0:V], M2[:, 0:V],
                                  ALU.mult)
                vec.tensor_tensor(Sp[:, 0:V], bp[:, 0:V], rd16[:, 0:V],
                                  ALU.mult)
                vec.tensor_tensor(Sm[:, 0:V], bm[:, 0:V], rd16[:, 0:V],
                                  ALU.mult)
                vec.tensor_tensor(dr[:, 0:V], Sp[:, 0:V], G2[:, 0:V], ALU.add)
                vec.tensor_tensor(csm[:, 0:V], cr[:, 0:V], Sm[:, 0:V],
                                  ALU.mult)
                vec.tensor_tensor(dm[:, 0:V], urr[:, 0:V], dr[:, 0:V],
                                  ALU.mult)
                vec.tensor_tensor(dm[:, 0:V], dm[:, 0:V], csm[:, 0:V], ALU.add)
                # DE2 = 2*de = D2*Sp + ur*(dm+csm)
                vec.tensor_tensor(dSp[:, 0:V], D2[:, 0:V], Sp[:, 0:V],
                                  ALU.mult)
                vec.tensor_tensor(DE2[:, 0:V], dm[:, 0:V], csm[:, 0:V],
                                  ALU.add)
                vec.tensor_tensor(DE2[:, 0:V], urr[:, 0:V], DE2[:, 0:V],
                                  ALU.mult)
                vec.tensor_tensor(DE2[:, 0:V], dSp[:, 0:V], DE2[:, 0:V],
                                  ALU.add)

                # gps: central-flux differences (fp32 in -> fp16 out)
                gps.tensor_tensor(gg[0][:, 0:UPD], mu[:, 2:W], mu[:, 0:UPD],
                                  ALU.subtract)
                gps.tensor_tensor(gg[1][:, 0:UPD], Fm[:, 2:W], Fm[:, 0:UPD],
                                  ALU.subtract)
                gps.tensor_tensor(gg[2][:, 0:UPD], Fe[:, 2:W], Fe[:, 0:UPD],
                                  ALU.subtract)

                # ---- dt chain (consumes the AllReduce result) ----
                gps.partition_all_reduce(small["gball"][:], small["gpp"][:],
                                         channels=P,
                                         reduce_op=bass_isa.ReduceOp.max)
                vec.reciprocal_approx_accurate(small["rgi"][:],
                                               small["gball"][:],
                                               small["rgs"][:])
                vec.tensor_scalar_mul(small["dt0"][:], small["rgi"][:],
                                      float(CFL * DX))
                vec.scalar_tensor_tensor(small["rem"][:], small["tcur"][:],
                                         -1.0, small["tfb"][:],
                                         ALU.mult, ALU.add)
                vec.tensor_scalar_max(small["rem"][:], small["rem"][:], 0.0)
                vec.tensor_tensor(small["dtt"][:], small["dt0"][:],
                                  small["rem"][:], ALU.min)
                vec.tensor_tensor(small["tcur"][:], small["tcur"][:],
                                  small["dtt"][:], ALU.add)
                vec.tensor_scalar_mul(small["hdtn"][:], small["dtt"][:],
                                      float(-0.5 / DX))

                # ---- update: state += hdtn*(dcF - dd) ----
                for gi, (diss, st, half) in enumerate(
                        ((dr, rho, False), (dm, mu, False), (DE2, En, True))):
                    vec.tensor_tensor(ddt[:, 0:UPD], diss[:, 1:V],
                                      diss[:, 0:V - 1], ALU.subtract)
                    if half:
                        # diss_E is 2*de; fold the 1/2 here
                        vec.scalar_tensor_tensor(gg[gi][:, 0:UPD],
                                                 ddt[:, 0:UPD], -0.5,
                                                 gg[gi][:, 0:UPD],
                                                 ALU.mult, ALU.add)
                    else:
                        vec.tensor_tensor(gg[gi][:, 0:UPD], gg[gi][:, 0:UPD],
                                          ddt[:, 0:UPD], ALU.subtract)
                    vec.scalar_tensor_tensor(st[:, 1:W - 1], gg[gi][:, 0:UPD],
                                             small["hdtn"][:], st[:, 1:W - 1],
                                             ALU.mult, ALU.add)

            # ---- epilogue: final u, p on own cells ----
            own = slice(G, G + FPC)
            vec.reciprocal_approx_fast(rinv[:], rho[:])
            vec.tensor_tensor(uu[:], mu[:], rinv[:], ALU.mult)
            vec.tensor_tensor(q[:], mu[:], uu[:], ALU.mult)
            vec.scalar_tensor_tensor(p04[:], q[:], -0.5, En[:],
                                     ALU.mult, ALU.add)
            vec.tensor_scalar_mul(pp[:], p04[:], 0.4)
            nc.sync.dma_start(out=rho_out.ap(), in_=rho[:, own])
            nc.sync.dma_start(out=u_out.ap(), in_=uu[:, own])
            nc.sync.dma_start(out=p_out.ap(), in_=pp[:, own])

    nc.compile()
    return nc


def _get_program(n_steps: int):
    if n_steps not in _CACHE:
        _CACHE[n_steps] = _build(n_steps)
    return _CACHE[n_steps]


def kernel(rho_init, u_init, p_init, t_final, n_steps):
    rho_init = np.ascontiguousarray(np.asarray(rho_init, np.float32))
    u_init = np.ascontiguousarray(np.asarray(u_init, np.float32))
    p_init = np.ascontiguousarray(np.asarray(p_init, np.float32))
    tf = np.float32(np.asarray(t_final).reshape(()))
    ns = int(np.asarray(n_steps).reshape(()))
    assert rho_init.shape == (NX,)

    gm1 = np.float32(GAMMA - 1.0)
    cells = NX // NC
    idx = (np.arange(P)[:, None] * FPC) + (np.arange(W)[None, :] - G)

    in_maps = []
    for k in range(NC):
        gi = np.clip(k * cells + idx, 0, NX - 1)
        r = rho_init[gi]
        u = u_init[gi]
        p = p_init[gi]
        mu_ = r * u
        E = p / gm1 + np.float32(0.5) * r * u * u
        mskL = np.zeros((P, G), np.uint8)
        mskR = np.zeros((P, G), np.uint8)
        if k == 0:
            mskL[0, :] = 1
        if k == NC - 1:
            mskR[P - 1, :] = 1
        in_maps.append({
            "rho_in": np.ascontiguousarray(r),
            "mu_in": np.ascontiguousarray(mu_),
            "E_in": np.ascontiguousarray(E),
            "tf_in": np.full((1, 1), tf, np.float32),
            "mskL_in": mskL,
            "mskR_in": mskR,
        })

    nc = _get_program(ns)
    res = run_bass_kernel_spmd(nc, in_maps, core_ids=list(range(NC)))
    global _last_results
    _last_results = res

    rho_o = np.empty(NX, np.float32)
    u_o = np.empty(NX, np.float32)
    p_o = np.empty(NX, np.float32)
    for k in range(NC):
        sl = slice(k * cells, (k + 1) * cells)
        rho_o[sl] = res.results[k]["rho_out"].reshape(-1)
        u_o[sl] = res.results[k]["u_out"].reshape(-1)
        p_o[sl] = res.results[k]["p_out"].reshape(-1)
    return rho_o, u_o, p_o
